# revision 6
# baseline (speedup 1.0000x reference)
"""Trainium2 Bass kernel for 16-head attention (B=4, S=2048, D=1024).

Sharding: 8 cores = 4 batches x 2 head-groups. Core c handles batch c//2,
heads (c%2)*8 .. +8. Each core computes a partial projection output
[S, D]; the host sums the two head-group partials per batch and adds
b_proj. No collectives.

v2: single software-pipelined phase. The attention (hp, n) windows start
as soon as head-pair 0's K/Q chunk and the first V tiles exist; the
remaining QKV projections and the output projection are emitted as
"filler" units pumped between attention j-iterations, so the Scalar
engine (exp, the pacing engine at ~1.02us per [128,1024] tile) never
idles waiting for a phase boundary. The softmax reciprocal is computed
on denominators DMA-spread across 128 partitions ([1,1024]->[128,8]),
turning a 4.3us single-partition iterative divide into ~0.2us.

Per-core layout trick: host feeds x[b] transposed (xT [D, S]), so the QKV
matmuls produce Q^T / K^T in [qkv-col, seq] layout directly, scores are
computed transposed ([sk, sq]) and softmax is done without max-subtraction
(inputs are bounded; exp stays well inside fp32/bf16 range). V is
ones-augmented so the attn@V matmul also yields softmax row-sums for free;
normalization uses the spread reciprocal + a K=1 outer-product matmul to
broadcast the per-column scale across partitions.
"""

import sys
import os

sys.path.insert(0, "/opt/trn_rl_repo")

import numpy as np
import ml_dtypes

BF = ml_dtypes.bfloat16

DIM = 1024
N_HEADS = 16
HD = 64
B = 4
S = 2048
HPC = 8          # heads per core
GC = HPC * HD    # 512 columns per head-group
N_CORES = 8
SCALE = HD ** -0.5

_CACHE = {}


def _build_bass():
    import concourse.bass as bass
    import concourse.mybir as mybir
    import concourse.tile as tile
    from concourse import bacc

    f32 = mybir.dt.float32
    bf16 = mybir.dt.bfloat16
    EXP = mybir.ActivationFunctionType.Exp

    nc = bacc.Bacc("TRN2", target_bir_lowering=False, debug=False,
                   num_devices=N_CORES)

    xT = nc.dram_tensor("xT", [DIM, S], bf16, kind="ExternalInput").ap()
    wq = nc.dram_tensor("wq", [DIM, GC], bf16, kind="ExternalInput").ap()
    wk = nc.dram_tensor("wk", [DIM, GC], bf16, kind="ExternalInput").ap()
    wv = nc.dram_tensor("wv", [DIM, GC], bf16, kind="ExternalInput").ap()
    wp = nc.dram_tensor("wp", [GC, DIM], bf16, kind="ExternalInput").ap()
    # q/k biases pre-broadcast on host: [128, m-tile*1024], each m block
    # holds the per-partition bias value replicated over 2x512 columns
    bq = nc.dram_tensor("bq", [128, 4096], f32, kind="ExternalInput").ap()
    bk = nc.dram_tensor("bk", [128, 4096], f32, kind="ExternalInput").ap()
    bvb = nc.dram_tensor("bvb", [128, GC], f32, kind="ExternalInput").ap()
    out = nc.dram_tensor("out", [S, DIM], f32, kind="ExternalOutput").ap()

    KD = DIM // 128   # 8 k-tiles over D
    NQ = GC // 128    # 4 tiles over the 512 head-group columns
    NS = S // 512     # 4 seq chunks of 512
    ST = S // 128     # 16 seq tiles of 128

    with tile.TileContext(nc) as tc:
        with tc.tile_pool(name="const", bufs=1) as cp:
            # input DMAs in consumption order: attention for head-pair 0
            # needs xT + wk + wq first; V weights next; proj weights and
            # biases are consumed late.
            xTs, wqs, wks, wvs = [], [], [], []
            for k in range(KD):
                t = cp.tile([128, S], bf16, name=f"wxs{k}")
                nc.sync.dma_start(t[:], xT[k * 128:(k + 1) * 128, :])
                xTs.append(t)
                for lst, src, nm in ((wks, wk, "k"), (wqs, wq, "q")):
                    t = cp.tile([128, GC], bf16, name=f"w{nm}s{k}")
                    nc.sync.dma_start(t[:], src[k * 128:(k + 1) * 128, :])
                    lst.append(t)
            for k in range(KD):
                t = cp.tile([128, GC], bf16, name=f"wvs{k}")
                nc.sync.dma_start(t[:], wv[k * 128:(k + 1) * 128, :])
                wvs.append(t)
            wps = []
            for k in range(NQ):
                t = cp.tile([128, DIM], bf16, name=f"wps{k}")
                nc.sync.dma_start(t[:], wp[k * 128:(k + 1) * 128, :])
                wps.append(t)
            bq_sb = cp.tile([128, 4096], f32, name="bq_sb")
            nc.sync.dma_start(bq_sb[:], bq[:, :])
            bk_sb = cp.tile([128, 4096], f32, name="bk_sb")
            nc.sync.dma_start(bk_sb[:], bk[:, :])
            bvb_sb = cp.tile([128, GC], f32, name="bvb_sb")
            nc.sync.dma_start(bvb_sb[:], bvb[:, :])
            ones_sb = cp.tile([128, 64], bf16, name="ones_sb")
            nc.any.memset(ones_sb[:], 1.0)

            QT = [cp.tile([128, S], bf16, name=f"QT{m}") for m in range(NQ)]
            KT = [cp.tile([128, S], bf16, name=f"KT{m}") for m in range(NQ)]
            # V tiles: per head 65 cols (64 data + trailing ones column)
            Vt = [cp.tile([128, HPC * 65], bf16, name=f"Vt{s}")
                  for s in range(ST)]
            OT = [cp.tile([128, S], bf16, name=f"OT{m}") for m in range(NQ)]

            for s in range(ST):
                ones_cols = Vt[s][:, :].rearrange(
                    "p (h c) -> p h c", c=65)[:, :, 64:65]
                nc.any.memset(ones_cols, 1.0)

            # PSUM: "s" 2x[128,1024] (scores ping-pong, 4 banks),
            # "f" 1x[128,1024] (QKV/proj fillers + normalize broadcast,
            # 2 banks), "o" 2x[128,512] (attn@V accumulators, 2 banks).
            with tc.tile_pool(name="ps", bufs=1, space="PSUM") as psp, \
                 tc.tile_pool(name="pbuf", bufs=4) as pbufp, \
                 tc.tile_pool(name="un", bufs=4) as unp, \
                 tc.tile_pool(name="dn", bufs=3) as dnp, \
                 tc.tile_pool(name="stg", bufs=3) as stgp:

                def ps_f(name, tag="f"):
                    return psp.tile([128, 1024], f32, tag=tag,
                                    bufs=2 if tag == "s" else 1, name=name)

                # ---- filler units (one PSUM tile each; prologue units
                # run on the idle "s" ring so they pipeline, attention-
                # phase fillers on the single "f" slot) ----
                def qk_unit(dst, ws, bias, m, n2, nm, tag="f"):
                    def emit():
                        ps = ps_f(f"{nm}{m}{n2}", tag)
                        for k in range(KD):
                            for h in range(2):
                                nc.tensor.matmul(
                                    ps[:, h * 512:(h + 1) * 512],
                                    lhsT=ws[k][:, m * 128:(m + 1) * 128],
                                    rhs=xTs[k][:, (n2 * 2 + h) * 512:
                                               (n2 * 2 + h + 1) * 512],
                                    start=(k == 0), stop=(k == KD - 1))
                        nc.vector.tensor_add(
                            dst[m][:, n2 * 1024:(n2 + 1) * 1024], ps[:],
                            bias[:, m * 1024:(m + 1) * 1024])
                    return emit

                def v_unit(s2, tag="f"):
                    def emit():
                        ps = ps_f(f"v{s2}", tag)
                        for k in range(KD):
                            for h in range(2):
                                st = (s2 * 2 + h) * 128
                                nc.tensor.matmul(
                                    ps[:, h * 512:(h + 1) * 512],
                                    lhsT=xTs[k][:, st:st + 128],
                                    rhs=wvs[k][:, :],
                                    start=(k == 0), stop=(k == KD - 1))
                        for h in range(2):
                            src3 = ps[:, h * 512:(h + 1) * 512].rearrange(
                                "p (g c) -> p g c", c=64)
                            bv3 = bvb_sb[:].rearrange("p (g c) -> p g c",
                                                      c=64)
                            dst3 = Vt[s2 * 2 + h][:, :].rearrange(
                                "p (g c) -> p g c", c=65)[:, :, 0:64]
                            nc.vector.tensor_add(dst3, src3, bv3)
                    return emit

                def proj_unit(m):
                    def emit():
                        ps = ps_f(f"pj{m}")
                        for k in range(NQ):
                            for h in range(2):
                                nc.tensor.matmul(
                                    ps[:, h * 512:(h + 1) * 512],
                                    lhsT=OT[k][:, m * 128:(m + 1) * 128],
                                    rhs=wps[k][:, h * 512:(h + 1) * 512],
                                    start=(k == 0), stop=(k == NQ - 1))
                        ob = stgp.tile([128, 1024], f32, tag="ob",
                                       name=f"ob{m}")
                        nc.vector.tensor_copy(ob[:], ps[:])
                        nc.sync.dma_start(out[m * 128:(m + 1) * 128, :],
                                          ob[:])
                    return emit

                fillq = []

                def pump():
                    if fillq:
                        fillq.pop(0)()

                # ---- normalize (deferred by one attention window) ----
                def emit_normalize(p):
                    hp, n, us = p
                    sq = slice(n * 512, (n + 1) * 512)
                    # spread both heads' denominators across partitions,
                    # reciprocal there, gather back to one row
                    dsp = dnp.tile([128, 8], f32, tag="d",
                                   name=f"d{hp}{n}")
                    nc.sync.dma_start(dsp[0:64, :], us[0][64:65, 0:512])
                    nc.sync.dma_start(dsp[64:128, :], us[1][64:65, 0:512])
                    rsp = dnp.tile([128, 8], bf16, tag="rs",
                                   name=f"rs{hp}{n}")
                    with nc.allow_low_precision(
                            reason="bf16 softmax denom matches bf16 "
                                   "matmul precision"):
                        nc.vector.reciprocal(rsp[:], dsp[:])
                    rT = dnp.tile([1, 1024], bf16, tag="rt",
                                  name=f"rt{hp}{n}")
                    nc.sync.dma_start(rT[0:1, 0:512], rsp[0:64, :])
                    nc.sync.dma_start(rT[0:1, 512:1024], rsp[64:128, :])
                    pb = ps_f(f"pb{hp}{n}")
                    for half in range(2):
                        nc.tensor.matmul(
                            pb[0:64, half * 512:(half + 1) * 512],
                            lhsT=ones_sb[0:1, 0:64],
                            rhs=rT[0:1, half * 512:(half + 1) * 512],
                            start=True, stop=True)
                    nc.vector.tensor_mul(OT[hp][0:64, sq], us[0][0:64, :],
                                         pb[0:64, 0:512])
                    stB = stgp.tile([64, 512], bf16, tag="st",
                                    name=f"stB{hp}{n}")
                    nc.vector.tensor_mul(stB[:], us[1][0:64, :],
                                         pb[0:64, 512:1024])
                    nc.sync.dma_start(OT[hp][64:128, sq], stB[:])

                # ---- prologue: K/Q for head-pair 0, all V ----
                qk_unit(KT, wks, bk_sb, 0, 0, "K", "s")()
                qk_unit(QT, wqs, bq_sb, 0, 0, "Q", "s")()
                v_unit(0, "s")()
                qk_unit(KT, wks, bk_sb, 0, 1, "K", "s")()
                for s2 in range(1, ST // 2):
                    v_unit(s2, "s")()

                # remaining QKV as fillers (K/Q for hp before its first
                # window; second Q chunks before n=2)
                for hp in range(1, NQ):
                    fillq.append(qk_unit(KT, wks, bk_sb, hp, 0, "K"))
                    fillq.append(qk_unit(KT, wks, bk_sb, hp, 1, "K"))
                    fillq.append(qk_unit(QT, wqs, bq_sb, hp, 0, "Q"))
                for hp in range(NQ):
                    fillq.append(qk_unit(QT, wqs, bq_sb, hp, 1, "Q"))

                # ---- attention windows ----
                pending = None
                for n in range(NS):
                    sq = slice(n * 512, (n + 1) * 512)
                    for hp in range(NQ):
                        oA = psp.tile([128, 512], f32, tag="o", bufs=2,
                                      name=f"oA{hp}{n}")
                        oB = psp.tile([128, 512], f32, tag="o", bufs=2,
                                      name=f"oB{hp}{n}")
                        for j in range(ST):
                            sk = slice(j * 128, (j + 1) * 128)
                            sS = psp.tile([128, 1024], f32, tag="s",
                                          bufs=2, name=f"sS{hp}{n}{j}")
                            nc.tensor.matmul(
                                sS[:, 0:512], lhsT=KT[hp][0:64, sk],
                                rhs=QT[hp][0:64, sq],
                                start=True, stop=True)
                            nc.tensor.matmul(
                                sS[:, 512:1024], lhsT=KT[hp][64:128, sk],
                                rhs=QT[hp][64:128, sq],
                                start=True, stop=True)
                            pT = pbufp.tile([128, 1024], bf16, tag="p",
                                            name=f"pT{hp}{n}{j}")
                            nc.scalar.activation(pT[:], sS[:], EXP,
                                                 scale=SCALE)
                            ha = hp * 2
                            nc.tensor.matmul(
                                oA[0:65, :],
                                lhsT=Vt[j][:, ha * 65:ha * 65 + 65],
                                rhs=pT[:, 0:512],
                                start=(j == 0), stop=(j == ST - 1))
                            nc.tensor.matmul(
                                oB[0:65, :],
                                lhsT=Vt[j][:, ha * 65 + 65:ha * 65 + 130],
                                rhs=pT[:, 512:1024],
                                start=(j == 0), stop=(j == ST - 1))
                            if j == 1 and pending is not None:
                                emit_normalize(pending)
                                pending = None
                                # after (n,3)'s normalize: proj of chunk
                                # n-1 is fully unblocked
                                if hp == 0 and n > 0:
                                    for m in range(4 * (n - 1),
                                                   4 * n):
                                        fillq.append(proj_unit(m))
                            elif j in (5, 9, 13):
                                pump()
                        us = []
                        for half, oPS in ((0, oA), (1, oB)):
                            u = unp.tile([128, 512], f32, tag="u",
                                         name=f"u{hp}{n}{half}")
                            nc.vector.tensor_copy(u[0:65, :],
                                                  oPS[0:65, :])
                            us.append(u)
                        pending = (hp, n, us)

                # ---- epilogue ----
                emit_normalize(pending)
                while fillq:
                    pump()
                for m in range(4 * (NS - 1), 4 * NS):
                    proj_unit(m)()
    nc.compile()
    return nc


def _get_nc():
    if "nc" not in _CACHE:
        _CACHE["nc"] = _build_bass()
    return _CACHE["nc"]


def _in_maps(x, w_qkv, b_qkv, w_proj, b_proj):
    x = np.asarray(x, np.float32)
    w_qkv = np.asarray(w_qkv, np.float32)
    b_qkv = np.asarray(b_qkv, np.float32)
    w_proj = np.asarray(w_proj, np.float32)

    def bias_bcast(b512):
        # [128, 4096]: m-tile blocks of 1024 cols, value per partition
        col = b512.reshape(4, 128).T[:, :, None]            # [128, 4, 1]
        return np.ascontiguousarray(
            np.broadcast_to(col, (128, 4, 1024)).reshape(128, 4096))

    maps = []
    for c in range(N_CORES):
        b, g = divmod(c, 2)
        cols = slice(g * GC, (g + 1) * GC)
        wqs = w_qkv[:, 0 * DIM:1 * DIM][:, cols]
        wks = w_qkv[:, 1 * DIM:2 * DIM][:, cols]
        wvs = w_qkv[:, 2 * DIM:3 * DIM][:, cols]
        bqs = b_qkv[0 * DIM:1 * DIM][cols]
        bks = b_qkv[1 * DIM:2 * DIM][cols]
        bvs = b_qkv[2 * DIM:3 * DIM][cols]
        rows = slice(g * GC, (g + 1) * GC)
        maps.append({
            "xT": np.ascontiguousarray(x[b].T).astype(BF),
            "wq": wqs.astype(BF),
            "wk": wks.astype(BF),
            "wv": wvs.astype(BF),
            "wp": w_proj[rows, :].astype(BF),
            "bq": bias_bcast(bqs),
            "bk": bias_bcast(bks),
            "bvb": np.broadcast_to(bvs, (128, GC)).copy(),
        })
    return maps


def kernel(x, w_qkv, b_qkv, w_proj, b_proj, _trace=False):
    import time
    from concourse import bass_utils
    nc = _get_nc()
    maps = _in_maps(x, w_qkv, b_qkv, w_proj, b_proj)
    try:
        res = bass_utils.run_bass_kernel_spmd(nc, maps,
                                              core_ids=list(range(N_CORES)),
                                              trace=_trace)
    except Exception:
        # a previously wedged device usually clears after one failed
        # attempt; retry once
        time.sleep(5)
        res = bass_utils.run_bass_kernel_spmd(nc, maps,
                                              core_ids=list(range(N_CORES)),
                                              trace=_trace)
    _CACHE["last_result"] = res
    b_proj = np.asarray(b_proj, np.float32)
    outs = np.empty((B, S, DIM), np.float32)
    for b in range(B):
        outs[b] = (res.results[2 * b]["out"] + res.results[2 * b + 1]["out"]
                   + b_proj)
    return outs


# revision 8
# speedup vs baseline: 1.0180x; 1.0180x over previous
"""Trainium2 Bass kernel for 16-head attention (B=4, S=2048, D=1024).

Sharding: 8 cores = 4 batches x 2 head-groups. Core c handles batch c//2,
heads (c%2)*8 .. +8. Each core computes a partial projection output
[S, D]; the host sums the two head-group partials per batch and adds
b_proj. No collectives.

v2: single software-pipelined phase. The attention (hp, n) windows start
as soon as head-pair 0's K/Q chunk and the first V tiles exist; the
remaining QKV projections and the output projection are emitted as
"filler" units pumped between attention j-iterations, so the Scalar
engine (exp, the pacing engine at ~1.02us per [128,1024] tile) never
idles waiting for a phase boundary. The softmax reciprocal is computed
on denominators DMA-spread across 128 partitions ([1,1024]->[128,8]),
turning a 4.3us single-partition iterative divide into ~0.2us.

Per-core layout trick: host feeds x[b] transposed (xT [D, S]), so the QKV
matmuls produce Q^T / K^T in [qkv-col, seq] layout directly, scores are
computed transposed ([sk, sq]) and softmax is done without max-subtraction
(inputs are bounded; exp stays well inside fp32/bf16 range). V is
ones-augmented so the attn@V matmul also yields softmax row-sums for free;
normalization uses the spread reciprocal + a K=1 outer-product matmul to
broadcast the per-column scale across partitions.
"""

import sys
import os

sys.path.insert(0, "/opt/trn_rl_repo")

import numpy as np
import ml_dtypes

BF = ml_dtypes.bfloat16

DIM = 1024
N_HEADS = 16
HD = 64
B = 4
S = 2048
HPC = 8          # heads per core
GC = HPC * HD    # 512 columns per head-group
N_CORES = 8
SCALE = HD ** -0.5

_CACHE = {}


def _build_bass():
    import concourse.bass as bass
    import concourse.mybir as mybir
    import concourse.tile as tile
    from concourse import bacc

    f32 = mybir.dt.float32
    bf16 = mybir.dt.bfloat16
    EXP = mybir.ActivationFunctionType.Exp

    nc = bacc.Bacc("TRN2", target_bir_lowering=False, debug=False,
                   num_devices=N_CORES)

    xT = nc.dram_tensor("xT", [DIM, S], bf16, kind="ExternalInput").ap()
    wq = nc.dram_tensor("wq", [DIM, GC], bf16, kind="ExternalInput").ap()
    wk = nc.dram_tensor("wk", [DIM, GC], bf16, kind="ExternalInput").ap()
    wv = nc.dram_tensor("wv", [DIM, GC], bf16, kind="ExternalInput").ap()
    wp = nc.dram_tensor("wp", [GC, DIM], bf16, kind="ExternalInput").ap()
    # q/k biases pre-broadcast on host: [128, m-tile*1024], each m block
    # holds the per-partition bias value replicated over 2x512 columns
    bq = nc.dram_tensor("bq", [128, 4096], f32, kind="ExternalInput").ap()
    bk = nc.dram_tensor("bk", [128, 4096], f32, kind="ExternalInput").ap()
    bvb = nc.dram_tensor("bvb", [128, GC], f32, kind="ExternalInput").ap()
    out = nc.dram_tensor("out", [S, DIM], f32, kind="ExternalOutput").ap()

    KD = DIM // 128   # 8 k-tiles over D
    NQ = GC // 128    # 4 tiles over the 512 head-group columns
    NS = S // 512     # 4 seq chunks of 512
    ST = S // 128     # 16 seq tiles of 128

    with tile.TileContext(nc) as tc:
        with tc.tile_pool(name="const", bufs=1) as cp:
            # input DMAs in consumption order: attention for head-pair 0
            # needs xT + wk + wq first; V weights next; proj weights and
            # biases are consumed late.
            xTs, wqs, wks, wvs = [], [], [], []
            for k in range(KD):
                t = cp.tile([128, S], bf16, name=f"wxs{k}")
                nc.sync.dma_start(t[:], xT[k * 128:(k + 1) * 128, :])
                xTs.append(t)
                for lst, src, nm in ((wks, wk, "k"), (wqs, wq, "q")):
                    t = cp.tile([128, GC], bf16, name=f"w{nm}s{k}")
                    nc.sync.dma_start(t[:], src[k * 128:(k + 1) * 128, :])
                    lst.append(t)
            for k in range(KD):
                t = cp.tile([128, GC], bf16, name=f"wvs{k}")
                nc.sync.dma_start(t[:], wv[k * 128:(k + 1) * 128, :])
                wvs.append(t)
            wps = []
            for k in range(NQ):
                t = cp.tile([128, DIM], bf16, name=f"wps{k}")
                nc.sync.dma_start(t[:], wp[k * 128:(k + 1) * 128, :])
                wps.append(t)
            bq_sb = cp.tile([128, 4096], f32, name="bq_sb")
            nc.sync.dma_start(bq_sb[:], bq[:, :])
            bk_sb = cp.tile([128, 4096], f32, name="bk_sb")
            nc.sync.dma_start(bk_sb[:], bk[:, :])
            bvb_sb = cp.tile([128, GC], f32, name="bvb_sb")
            nc.sync.dma_start(bvb_sb[:], bvb[:, :])
            ones_sb = cp.tile([128, 64], bf16, name="ones_sb")
            nc.any.memset(ones_sb[:], 1.0)

            QT = [cp.tile([128, S], bf16, name=f"QT{m}") for m in range(NQ)]
            KT = [cp.tile([128, S], bf16, name=f"KT{m}") for m in range(NQ)]
            # V tiles: per head 65 cols (64 data + trailing ones column)
            Vt = [cp.tile([128, HPC * 65], bf16, name=f"Vt{s}")
                  for s in range(ST)]
            OT = [cp.tile([128, S], bf16, name=f"OT{m}") for m in range(NQ)]

            for s in range(ST):
                ones_cols = Vt[s][:, :].rearrange(
                    "p (h c) -> p h c", c=65)[:, :, 64:65]
                nc.any.memset(ones_cols, 1.0)

            # PSUM: "s" 2x[128,1024] (scores, two-ahead ping-pong,
            # 4 banks), "f" 2x[128,512] (QKV/proj filler half-units +
            # normalize broadcast, 2 banks), "o" 2x[128,512] (attn@V
            # accumulators, 2 banks).
            with tc.tile_pool(name="ps", bufs=1, space="PSUM") as psp, \
                 tc.tile_pool(name="pbuf", bufs=4) as pbufp, \
                 tc.tile_pool(name="un", bufs=4) as unp, \
                 tc.tile_pool(name="dn", bufs=3) as dnp, \
                 tc.tile_pool(name="stg", bufs=3) as stgp:

                def ps_f(name):
                    return psp.tile([128, 512], f32, tag="f", bufs=2,
                                    name=name)

                # ---- filler half-units (8 matmuls into one "f" tile;
                # emitted either as inline blocks at fixed window slots
                # or in 2-matmul steps via the generator pump) ----
                def qk_half(dst, ws, bias, m, c):
                    # c: 512-col seq chunk index (0..3)
                    def emit():
                        ps = ps_f(f"qk{m}{c}")
                        for k in range(KD):
                            nc.tensor.matmul(
                                ps[:],
                                lhsT=ws[k][:, m * 128:(m + 1) * 128],
                                rhs=xTs[k][:, c * 512:(c + 1) * 512],
                                start=(k == 0), stop=(k == KD - 1))
                            if k % 4 == 3:
                                yield
                        nc.vector.tensor_add(
                            dst[m][:, c * 512:(c + 1) * 512], ps[:],
                            bias[:, m * 1024:m * 1024 + 512])
                        yield
                    return emit

                def v_half(s):
                    def emit():
                        ps = ps_f(f"v{s}")
                        for k in range(KD):
                            nc.tensor.matmul(
                                ps[:], lhsT=xTs[k][:, s * 128:(s + 1) * 128],
                                rhs=wvs[k][:, :],
                                start=(k == 0), stop=(k == KD - 1))
                            if k % 4 == 3:
                                yield
                        src3 = ps[:].rearrange("p (g c) -> p g c", c=64)
                        bv3 = bvb_sb[:].rearrange("p (g c) -> p g c", c=64)
                        dst3 = Vt[s][:, :].rearrange(
                            "p (g c) -> p g c", c=65)[:, :, 0:64]
                        nc.vector.tensor_add(dst3, src3, bv3)
                        yield
                    return emit

                def proj_half(m, h):
                    def emit():
                        ps = ps_f(f"pj{m}{h}")
                        for k in range(NQ):
                            nc.tensor.matmul(
                                ps[:],
                                lhsT=OT[k][:, m * 128:(m + 1) * 128],
                                rhs=wps[k][:, h * 512:(h + 1) * 512],
                                start=(k == 0), stop=(k == NQ - 1))
                            if k == 1:
                                yield
                        ob = stgp.tile([128, 512], f32, tag="ob",
                                       name=f"ob{m}{h}")
                        nc.vector.tensor_copy(ob[:], ps[:])
                        nc.sync.dma_start(
                            out[m * 128:(m + 1) * 128,
                                h * 512:(h + 1) * 512], ob[:])
                        yield
                    return emit

                # generator pump: each step emits ~2 matmuls
                fillq = []
                state = {"cur": None}

                def pump():
                    while True:
                        if state["cur"] is None:
                            if not fillq:
                                return
                            state["cur"] = fillq.pop(0)()
                        try:
                            next(state["cur"])
                            return
                        except StopIteration:
                            state["cur"] = None

                def run_block(gen_fn):
                    # emit a half-unit contiguously (inline block)
                    for _ in gen_fn():
                        pass

                # ---- normalize, emitted at the end of each window ----
                def emit_normalize(hp, n, us):
                    sq = slice(n * 512, (n + 1) * 512)
                    dsp = dnp.tile([128, 8], f32, tag="d",
                                   name=f"d{hp}{n}")
                    nc.sync.dma_start(dsp[0:64, :], us[0][64:65, 0:512])
                    nc.sync.dma_start(dsp[64:128, :], us[1][64:65, 0:512])
                    rsp = dnp.tile([128, 8], bf16, tag="rs",
                                   name=f"rs{hp}{n}")
                    with nc.allow_low_precision(
                            reason="bf16 softmax denom matches bf16 "
                                   "matmul precision"):
                        nc.vector.reciprocal(rsp[:], dsp[:])
                    rT = dnp.tile([1, 1024], bf16, tag="rt",
                                  name=f"rt{hp}{n}")
                    nc.sync.dma_start(rT[0:1, 0:512], rsp[0:64, :])
                    nc.sync.dma_start(rT[0:1, 512:1024], rsp[64:128, :])
                    pbs = stgp.tile([64, 1024], f32, tag="pbs",
                                    name=f"pbs{hp}{n}")
                    for half in range(2):
                        pb = ps_f(f"pb{hp}{n}{half}")
                        nc.tensor.matmul(
                            pb[0:64, :], lhsT=ones_sb[0:1, 0:64],
                            rhs=rT[0:1, half * 512:(half + 1) * 512],
                            start=True, stop=True)
                        # copy to SBUF promptly so the "f" ring frees
                        # before the next window pumps fillers
                        nc.vector.tensor_copy(
                            pbs[:, half * 512:(half + 1) * 512],
                            pb[0:64, :])
                    nc.vector.tensor_mul(OT[hp][0:64, sq], us[0][0:64, :],
                                         pbs[:, 0:512])
                    stB = stgp.tile([64, 512], bf16, tag="st",
                                    name=f"stB{hp}{n}")
                    nc.vector.tensor_mul(stB[:], us[1][0:64, :],
                                         pbs[:, 512:1024])
                    nc.sync.dma_start(OT[hp][64:128, sq], stB[:])

                # inline blocks scheduled at fixed (window, j) slots:
                # window 0 produces Vt[4..15], K0's second half and
                # K1/Q1; windows 1-2 produce the rest of K/Q.
                inline = {}
                for i in range(12):          # Vt[4+i] at (0, j=i)
                    inline[(0, i)] = [v_half(4 + i)]

                def K(m, c):
                    return qk_half(KT, wks, bk_sb, m, c)

                def Q(m, c):
                    return qk_half(QT, wqs, bq_sb, m, c)

                # scores for body j are emitted at body j-2 and read K
                # chunk j//4, so K[hp] chunk c must be emitted before
                # body 4c-2 of window hp; Q[hp] chunk n before window
                # (n, hp) starts.
                sched = {
                    (0, 4): K(0, 2), (0, 8): K(0, 3),
                    (0, 12): K(1, 0), (0, 13): Q(1, 0), (0, 14): K(1, 1),
                    (1, 4): K(1, 2), (1, 6): K(1, 3),
                    (1, 8): K(2, 0), (1, 10): Q(2, 0),
                    (1, 12): K(2, 1), (1, 14): K(2, 2),
                    (2, 4): K(2, 3),
                    (2, 6): K(3, 0), (2, 8): Q(3, 0), (2, 10): K(3, 1),
                    (2, 12): K(3, 2), (2, 14): K(3, 3),
                    (3, 4): Q(0, 1), (3, 6): Q(1, 1),
                    (3, 8): Q(2, 1), (3, 10): Q(3, 1),
                }
                for key, u in sched.items():
                    inline.setdefault(key, []).append(u)
                # remaining Q chunks via the generator pump (needed from
                # window 8 / window 12 on; pumped during windows 4-7)
                for c in (2, 3):
                    for hp in range(NQ):
                        fillq.append(qk_half(QT, wqs, bq_sb, hp, c))

                # ---- prologue: K0/Q0 first chunks + Vt[0..3] ----
                run_block(qk_half(KT, wks, bk_sb, 0, 0))
                run_block(qk_half(KT, wks, bk_sb, 0, 1))
                run_block(qk_half(QT, wqs, bq_sb, 0, 0))
                run_block(qk_half(QT, wqs, bq_sb, 0, 1))
                for s in range(4):
                    run_block(v_half(s))

                # ---- attention windows ----
                for n in range(NS):
                    sq = slice(n * 512, (n + 1) * 512)
                    for hp in range(NQ):
                        w = n * NQ + hp
                        oA = psp.tile([128, 512], f32, tag="o", bufs=2,
                                      name=f"oA{hp}{n}")
                        oB = psp.tile([128, 512], f32, tag="o", bufs=2,
                                      name=f"oB{hp}{n}")

                        def emit_scores(j):
                            sk = slice(j * 128, (j + 1) * 128)
                            sS = psp.tile([128, 1024], f32, tag="s",
                                          bufs=2, name=f"sS{w}_{j}")
                            nc.tensor.matmul(
                                sS[:, 0:512], lhsT=KT[hp][0:64, sk],
                                rhs=QT[hp][0:64, sq],
                                start=True, stop=True)
                            nc.tensor.matmul(
                                sS[:, 512:1024],
                                lhsT=KT[hp][64:128, sk],
                                rhs=QT[hp][64:128, sq],
                                start=True, stop=True)
                            return sS

                        sS_q = [emit_scores(0), emit_scores(1)]
                        for j in range(ST):
                            for u in inline.get((w, j), []):
                                run_block(u)
                            pT = pbufp.tile([128, 1024], bf16, tag="p",
                                            name=f"pT{w}_{j}")
                            nc.scalar.activation(pT[:], sS_q[j][:], EXP,
                                                 scale=SCALE)
                            ha = hp * 2
                            nc.tensor.matmul(
                                oA[0:65, :],
                                lhsT=Vt[j][:, ha * 65:ha * 65 + 65],
                                rhs=pT[:, 0:512],
                                start=(j == 0), stop=(j == ST - 1))
                            nc.tensor.matmul(
                                oB[0:65, :],
                                lhsT=Vt[j][:, ha * 65 + 65:ha * 65 + 130],
                                rhs=pT[:, 512:1024],
                                start=(j == 0), stop=(j == ST - 1))
                            if j + 2 < ST:
                                sS_q.append(emit_scores(j + 2))
                            if w > 0 and j >= 4 and (w, j) not in inline:
                                pump()
                        us = []
                        for half, oPS in ((0, oA), (1, oB)):
                            u = unp.tile([128, 512], f32, tag="u",
                                         name=f"u{w}_{half}")
                            nc.vector.tensor_copy(u[0:65, :],
                                                  oPS[0:65, :])
                            us.append(u)
                        emit_normalize(hp, n, us)
                        if hp == NQ - 1:
                            for m in range(4 * n, 4 * n + 4):
                                fillq.append(proj_half(m, 0))
                                fillq.append(proj_half(m, 1))

                # ---- epilogue: drain remaining fillers (tail projs) ----
                while fillq or state["cur"] is not None:
                    pump()
    nc.compile()
    return nc


def _get_nc():
    if "nc" not in _CACHE:
        _CACHE["nc"] = _build_bass()
    return _CACHE["nc"]


def _in_maps(x, w_qkv, b_qkv, w_proj, b_proj):
    x = np.asarray(x, np.float32)
    w_qkv = np.asarray(w_qkv, np.float32)
    b_qkv = np.asarray(b_qkv, np.float32)
    w_proj = np.asarray(w_proj, np.float32)

    def bias_bcast(b512):
        # [128, 4096]: m-tile blocks of 1024 cols, value per partition
        col = b512.reshape(4, 128).T[:, :, None]            # [128, 4, 1]
        return np.ascontiguousarray(
            np.broadcast_to(col, (128, 4, 1024)).reshape(128, 4096))

    maps = []
    for c in range(N_CORES):
        b, g = divmod(c, 2)
        cols = slice(g * GC, (g + 1) * GC)
        wqs = w_qkv[:, 0 * DIM:1 * DIM][:, cols]
        wks = w_qkv[:, 1 * DIM:2 * DIM][:, cols]
        wvs = w_qkv[:, 2 * DIM:3 * DIM][:, cols]
        bqs = b_qkv[0 * DIM:1 * DIM][cols]
        bks = b_qkv[1 * DIM:2 * DIM][cols]
        bvs = b_qkv[2 * DIM:3 * DIM][cols]
        rows = slice(g * GC, (g + 1) * GC)
        maps.append({
            "xT": np.ascontiguousarray(x[b].T).astype(BF),
            "wq": wqs.astype(BF),
            "wk": wks.astype(BF),
            "wv": wvs.astype(BF),
            "wp": w_proj[rows, :].astype(BF),
            "bq": bias_bcast(bqs),
            "bk": bias_bcast(bks),
            "bvb": np.broadcast_to(bvs, (128, GC)).copy(),
        })
    return maps


def kernel(x, w_qkv, b_qkv, w_proj, b_proj, _trace=False):
    import time
    from concourse import bass_utils
    nc = _get_nc()
    maps = _in_maps(x, w_qkv, b_qkv, w_proj, b_proj)
    try:
        res = bass_utils.run_bass_kernel_spmd(nc, maps,
                                              core_ids=list(range(N_CORES)),
                                              trace=_trace)
    except Exception:
        # a previously wedged device usually clears after one failed
        # attempt; retry once
        time.sleep(5)
        res = bass_utils.run_bass_kernel_spmd(nc, maps,
                                              core_ids=list(range(N_CORES)),
                                              trace=_trace)
    _CACHE["last_result"] = res
    b_proj = np.asarray(b_proj, np.float32)
    outs = np.empty((B, S, DIM), np.float32)
    for b in range(B):
        outs[b] = (res.results[2 * b]["out"] + res.results[2 * b + 1]["out"]
                   + b_proj)
    return outs


# revision 11
# speedup vs baseline: 1.1007x; 1.0812x over previous
"""Trainium2 Bass kernel for 16-head attention (B=4, S=2048, D=1024).

Sharding: 8 cores = 4 batches x 2 head-groups. Core c handles batch c//2,
heads (c%2)*8 .. +8. Each core computes a partial projection output
[S, D]; the host sums the two head-group partials per batch and adds
b_proj. No collectives.

v2: single software-pipelined phase. The attention (hp, n) windows start
as soon as head-pair 0's K/Q chunk and the first V tiles exist; the
remaining QKV projections and the output projection are emitted as
"filler" units pumped between attention j-iterations, so the Scalar
engine (exp, the pacing engine at ~1.02us per [128,1024] tile) never
idles waiting for a phase boundary. The softmax reciprocal is computed
on denominators DMA-spread across 128 partitions ([1,1024]->[128,8]),
turning a 4.3us single-partition iterative divide into ~0.2us.

Per-core layout trick: host feeds x[b] transposed (xT [D, S]), so the QKV
matmuls produce Q^T / K^T in [qkv-col, seq] layout directly, scores are
computed transposed ([sk, sq]) and softmax is done without max-subtraction
(inputs are bounded; exp stays well inside fp32/bf16 range). V is
ones-augmented so the attn@V matmul also yields softmax row-sums for free;
normalization uses the spread reciprocal + a K=1 outer-product matmul to
broadcast the per-column scale across partitions.
"""

import sys
import os

sys.path.insert(0, "/opt/trn_rl_repo")

import numpy as np
import ml_dtypes

BF = ml_dtypes.bfloat16

DIM = 1024
N_HEADS = 16
HD = 64
B = 4
S = 2048
HPC = 8          # heads per core
GC = HPC * HD    # 512 columns per head-group
N_CORES = 8
SCALE = HD ** -0.5

_CACHE = {}


def _build_bass():
    import concourse.bass as bass
    import concourse.mybir as mybir
    import concourse.tile as tile
    from concourse import bacc

    f32 = mybir.dt.float32
    bf16 = mybir.dt.bfloat16
    EXP = mybir.ActivationFunctionType.Exp

    nc = bacc.Bacc("TRN2", target_bir_lowering=False, debug=False,
                   num_devices=N_CORES)

    xT = nc.dram_tensor("xT", [DIM, S], bf16, kind="ExternalInput").ap()
    wq = nc.dram_tensor("wq", [DIM, GC], bf16, kind="ExternalInput").ap()
    wk = nc.dram_tensor("wk", [DIM, GC], bf16, kind="ExternalInput").ap()
    wv = nc.dram_tensor("wv", [DIM, GC], bf16, kind="ExternalInput").ap()
    wp = nc.dram_tensor("wp", [GC, DIM], bf16, kind="ExternalInput").ap()
    # q/k biases as per-partition columns: [128, 4] (one col per m-tile);
    # applied via tensor_scalar_add's per-partition scalar operand
    bq = nc.dram_tensor("bq", [128, 4], f32, kind="ExternalInput").ap()
    bk = nc.dram_tensor("bk", [128, 4], f32, kind="ExternalInput").ap()
    bvb = nc.dram_tensor("bvb", [128, GC], f32, kind="ExternalInput").ap()
    out = nc.dram_tensor("out", [S, DIM], f32, kind="ExternalOutput").ap()

    KD = DIM // 128   # 8 k-tiles over D
    NQ = GC // 128    # 4 tiles over the 512 head-group columns
    NS = S // 512     # 4 seq chunks of 512
    ST = S // 128     # 16 seq tiles of 128

    with tile.TileContext(nc) as tc:
        with tc.tile_pool(name="const", bufs=1) as cp:
            # input DMAs in consumption order: attention for head-pair 0
            # needs xT + wk + wq first; V weights next; proj weights and
            # biases are consumed late.
            xTs, wqs, wks, wvs = [], [], [], []
            for k in range(KD):
                t = cp.tile([128, S], bf16, name=f"wxs{k}")
                nc.sync.dma_start(t[:], xT[k * 128:(k + 1) * 128, :])
                xTs.append(t)
            # weights + biases go on the scalar engine's HWDGE queue so
            # they stream in parallel with xT on the sync queue
            for k in range(KD):
                for lst, srct, nm in ((wks, wk, "k"), (wqs, wq, "q")):
                    t = cp.tile([128, GC], bf16, name=f"w{nm}s{k}")
                    nc.scalar.dma_start(t[:], srct[k * 128:(k + 1) * 128, :])
                    lst.append(t)
            bq_sb = cp.tile([128, 4], f32, name="bq_sb")
            nc.scalar.dma_start(bq_sb[:], bq[:, :])
            bk_sb = cp.tile([128, 4], f32, name="bk_sb")
            nc.scalar.dma_start(bk_sb[:], bk[:, :])
            for k in range(KD):
                t = cp.tile([128, GC], bf16, name=f"wvs{k}")
                nc.scalar.dma_start(t[:], wv[k * 128:(k + 1) * 128, :])
                wvs.append(t)
            bvb_sb = cp.tile([128, GC], f32, name="bvb_sb")
            nc.scalar.dma_start(bvb_sb[:], bvb[:, :])
            wps = []
            for k in range(NQ):
                t = cp.tile([128, DIM], bf16, name=f"wps{k}")
                nc.scalar.dma_start(t[:], wp[k * 128:(k + 1) * 128, :])
                wps.append(t)
            ones_sb = cp.tile([128, 64], bf16, name="ones_sb")
            nc.any.memset(ones_sb[:], 1.0)

            QT = [cp.tile([128, S], bf16, name=f"QT{m}") for m in range(NQ)]
            KT = [cp.tile([128, S], bf16, name=f"KT{m}") for m in range(NQ)]
            # V tiles: per head 65 cols (64 data + trailing ones column)
            Vt = [cp.tile([128, HPC * 65], bf16, name=f"Vt{s}")
                  for s in range(ST)]
            OT = [cp.tile([128, S], bf16, name=f"OT{m}") for m in range(NQ)]

            for s in range(ST):
                ones_cols = Vt[s][:, :].rearrange(
                    "p (h c) -> p h c", c=65)[:, :, 64:65]
                nc.any.memset(ones_cols, 1.0)

            # PSUM: "s" 2x[128,1024] (scores, two-ahead ping-pong,
            # 4 banks), "f" 2x[128,512] (QKV/proj filler half-units +
            # normalize broadcast, 2 banks), "o" 2x[128,512] (attn@V
            # accumulators, 2 banks).
            with tc.tile_pool(name="ps", bufs=1, space="PSUM") as psp, \
                 tc.tile_pool(name="pbuf", bufs=4) as pbufp, \
                 tc.tile_pool(name="un", bufs=4) as unp, \
                 tc.tile_pool(name="dn", bufs=3) as dnp, \
                 tc.tile_pool(name="stg", bufs=3) as stgp:

                def ps_f(name):
                    return psp.tile([128, 512], f32, tag="f", bufs=2,
                                    name=name)

                # ---- filler half-units (8 matmuls into one "f" tile;
                # emitted either as inline blocks at fixed window slots
                # or in 2-matmul steps via the generator pump) ----
                def qk_half(dst, ws, bias, m, c):
                    # c: 512-col seq chunk index (0..3)
                    def emit():
                        ps = ps_f(f"qk{m}{c}")
                        for k in range(KD):
                            nc.tensor.matmul(
                                ps[:],
                                lhsT=ws[k][:, m * 128:(m + 1) * 128],
                                rhs=xTs[k][:, c * 512:(c + 1) * 512],
                                start=(k == 0), stop=(k == KD - 1))
                            yield
                        nc.vector.tensor_scalar_add(
                            dst[m][:, c * 512:(c + 1) * 512], ps[:],
                            bias[:, m:m + 1])
                        yield
                    return emit

                def v_half(s):
                    def emit():
                        ps = ps_f(f"v{s}")
                        for k in range(KD):
                            nc.tensor.matmul(
                                ps[:], lhsT=xTs[k][:, s * 128:(s + 1) * 128],
                                rhs=wvs[k][:, :],
                                start=(k == 0), stop=(k == KD - 1))
                            yield
                        src3 = ps[:].rearrange("p (g c) -> p g c", c=64)
                        bv3 = bvb_sb[:].rearrange("p (g c) -> p g c", c=64)
                        dst3 = Vt[s][:, :].rearrange(
                            "p (g c) -> p g c", c=65)[:, :, 0:64]
                        nc.vector.tensor_add(dst3, src3, bv3)
                        yield
                    return emit

                def proj_half(m, h):
                    def emit():
                        ps = ps_f(f"pj{m}{h}")
                        for k in range(NQ):
                            nc.tensor.matmul(
                                ps[:],
                                lhsT=OT[k][:, m * 128:(m + 1) * 128],
                                rhs=wps[k][:, h * 512:(h + 1) * 512],
                                start=(k == 0), stop=(k == NQ - 1))
                            yield
                        ob = stgp.tile([128, 512], f32, tag="ob",
                                       name=f"ob{m}{h}")
                        nc.vector.tensor_copy(ob[:], ps[:])
                        nc.sync.dma_start(
                            out[m * 128:(m + 1) * 128,
                                h * 512:(h + 1) * 512], ob[:])
                        yield
                    return emit

                # generator pump: each step emits ~2 matmuls
                fillq = []
                state = {"cur": None}

                def pump():
                    while True:
                        if state["cur"] is None:
                            if not fillq:
                                return
                            state["cur"] = fillq.pop(0)()
                        try:
                            next(state["cur"])
                            return
                        except StopIteration:
                            state["cur"] = None

                def run_block(gen_fn):
                    # emit a half-unit contiguously (inline block)
                    for _ in gen_fn():
                        pass

                # ---- normalize, emitted at the end of each window ----
                def emit_normalize(hp, n, us):
                    sq = slice(n * 512, (n + 1) * 512)
                    dsp = dnp.tile([128, 8], f32, tag="d",
                                   name=f"d{hp}{n}")
                    nc.sync.dma_start(dsp[0:64, :], us[0][64:65, 0:512])
                    nc.sync.dma_start(dsp[64:128, :], us[1][64:65, 0:512])
                    rsp = dnp.tile([128, 8], bf16, tag="rs",
                                   name=f"rs{hp}{n}")
                    with nc.allow_low_precision(
                            reason="bf16 softmax denom matches bf16 "
                                   "matmul precision"):
                        nc.vector.reciprocal(rsp[:], dsp[:])
                    rT = dnp.tile([1, 1024], bf16, tag="rt",
                                  name=f"rt{hp}{n}")
                    nc.sync.dma_start(rT[0:1, 0:512], rsp[0:64, :])
                    nc.sync.dma_start(rT[0:1, 512:1024], rsp[64:128, :])
                    pbs = stgp.tile([64, 1024], f32, tag="pbs",
                                    name=f"pbs{hp}{n}")
                    for half in range(2):
                        pb = ps_f(f"pb{hp}{n}{half}")
                        nc.tensor.matmul(
                            pb[0:64, :], lhsT=ones_sb[0:1, 0:64],
                            rhs=rT[0:1, half * 512:(half + 1) * 512],
                            start=True, stop=True)
                        # copy to SBUF promptly so the "f" ring frees
                        # before the next window pumps fillers
                        nc.vector.tensor_copy(
                            pbs[:, half * 512:(half + 1) * 512],
                            pb[0:64, :])
                    nc.vector.tensor_mul(OT[hp][0:64, sq], us[0][0:64, :],
                                         pbs[:, 0:512])
                    stB = stgp.tile([64, 512], bf16, tag="st",
                                    name=f"stB{hp}{n}")
                    nc.vector.tensor_mul(stB[:], us[1][0:64, :],
                                         pbs[:, 512:1024])
                    nc.sync.dma_start(OT[hp][64:128, sq], stB[:])

                # inline blocks scheduled at fixed (window, j) slots:
                # window 0 produces Vt[4..15], K0's second half and
                # K1/Q1; windows 1-2 produce the rest of K/Q.
                inline = {}
                for i in range(12):          # Vt[4+i] at (0, j=i)
                    inline[(0, i)] = [v_half(4 + i)]

                def K(m, c):
                    return qk_half(KT, wks, bk_sb, m, c)

                def Q(m, c):
                    return qk_half(QT, wqs, bq_sb, m, c)

                # scores for body j are emitted at body j-2 and read K
                # chunk j//4, so K[hp] chunk c must be emitted before
                # body 4c-2 of window hp; Q[hp] chunk n before window
                # (n, hp) starts.
                sched = {
                    (0, 4): K(0, 2), (0, 8): K(0, 3),
                    (0, 12): K(1, 0), (0, 13): Q(1, 0), (0, 14): K(1, 1),
                    (1, 4): K(1, 2), (1, 6): K(1, 3),
                    (1, 8): K(2, 0), (1, 10): Q(2, 0),
                    (1, 12): K(2, 1), (1, 14): K(2, 2),
                    (2, 4): K(2, 3),
                    (2, 6): K(3, 0), (2, 8): Q(3, 0), (2, 10): K(3, 1),
                    (2, 12): K(3, 2), (2, 14): K(3, 3),
                    (3, 4): Q(0, 1), (3, 6): Q(1, 1),
                    (3, 8): Q(2, 1), (3, 10): Q(3, 1),
                }
                for key, u in sched.items():
                    inline.setdefault(key, []).append(u)
                # remaining Q chunks via the generator pump (needed from
                # window 8 / window 12 on; pumped during windows 4-7)
                for c in (2, 3):
                    for hp in range(NQ):
                        fillq.append(qk_half(QT, wqs, bq_sb, hp, c))

                # ---- prologue: K0/Q0 first chunks + Vt[0..3] ----
                run_block(qk_half(KT, wks, bk_sb, 0, 0))
                run_block(qk_half(KT, wks, bk_sb, 0, 1))
                run_block(qk_half(QT, wqs, bq_sb, 0, 0))
                run_block(qk_half(QT, wqs, bq_sb, 0, 1))
                for s in range(4):
                    run_block(v_half(s))

                # ---- attention windows ----
                for n in range(NS):
                    sq = slice(n * 512, (n + 1) * 512)
                    for hp in range(NQ):
                        w = n * NQ + hp
                        oA = psp.tile([128, 512], f32, tag="o", bufs=2,
                                      name=f"oA{hp}{n}")
                        oB = psp.tile([128, 512], f32, tag="o", bufs=2,
                                      name=f"oB{hp}{n}")

                        def emit_scores(j):
                            sk = slice(j * 128, (j + 1) * 128)
                            sS = psp.tile([128, 1024], f32, tag="s",
                                          bufs=2, name=f"sS{w}_{j}")
                            nc.tensor.matmul(
                                sS[:, 0:512], lhsT=KT[hp][0:64, sk],
                                rhs=QT[hp][0:64, sq],
                                start=True, stop=True)
                            nc.tensor.matmul(
                                sS[:, 512:1024],
                                lhsT=KT[hp][64:128, sk],
                                rhs=QT[hp][64:128, sq],
                                start=True, stop=True)
                            return sS

                        sS_q = [emit_scores(0), emit_scores(1)]
                        for j in range(ST):
                            for u in inline.get((w, j), []):
                                run_block(u)
                            pT = pbufp.tile([128, 1024], bf16, tag="p",
                                            name=f"pT{w}_{j}")
                            nc.scalar.activation(pT[:], sS_q[j][:], EXP,
                                                 scale=SCALE)
                            ha = hp * 2
                            nc.tensor.matmul(
                                oA[0:65, :],
                                lhsT=Vt[j][:, ha * 65:ha * 65 + 65],
                                rhs=pT[:, 0:512],
                                start=(j == 0), stop=(j == ST - 1))
                            nc.tensor.matmul(
                                oB[0:65, :],
                                lhsT=Vt[j][:, ha * 65 + 65:ha * 65 + 130],
                                rhs=pT[:, 512:1024],
                                start=(j == 0), stop=(j == ST - 1))
                            if j + 2 < ST:
                                sS_q.append(emit_scores(j + 2))
                            if w > 0 and j >= 3 and (w, j) not in inline:
                                pump()
                        us = []
                        for half, oPS in ((0, oA), (1, oB)):
                            u = unp.tile([128, 512], f32, tag="u",
                                         name=f"u{w}_{half}")
                            nc.vector.tensor_copy(u[0:65, :],
                                                  oPS[0:65, :])
                            us.append(u)
                        emit_normalize(hp, n, us)
                        if hp == NQ - 1:
                            for m in range(4 * n, 4 * n + 4):
                                fillq.append(proj_half(m, 0))
                                fillq.append(proj_half(m, 1))

                # ---- epilogue: drain remaining fillers (tail projs) ----
                while fillq or state["cur"] is not None:
                    pump()
    nc.compile()
    return nc


def _get_nc():
    if "nc" not in _CACHE:
        _CACHE["nc"] = _build_bass()
    return _CACHE["nc"]


def _in_maps(x, w_qkv, b_qkv, w_proj, b_proj):
    x = np.asarray(x, np.float32)
    w_qkv = np.asarray(w_qkv, np.float32)
    b_qkv = np.asarray(b_qkv, np.float32)
    w_proj = np.asarray(w_proj, np.float32)

    def bias_cols(b512):
        # [128, 4]: per-partition bias value, one column per m-tile
        return np.ascontiguousarray(b512.reshape(4, 128).T)

    maps = []
    for c in range(N_CORES):
        b, g = divmod(c, 2)
        cols = slice(g * GC, (g + 1) * GC)
        wqs = w_qkv[:, 0 * DIM:1 * DIM][:, cols]
        wks = w_qkv[:, 1 * DIM:2 * DIM][:, cols]
        wvs = w_qkv[:, 2 * DIM:3 * DIM][:, cols]
        bqs = b_qkv[0 * DIM:1 * DIM][cols]
        bks = b_qkv[1 * DIM:2 * DIM][cols]
        bvs = b_qkv[2 * DIM:3 * DIM][cols]
        rows = slice(g * GC, (g + 1) * GC)
        maps.append({
            "xT": np.ascontiguousarray(x[b].T).astype(BF),
            "wq": wqs.astype(BF),
            "wk": wks.astype(BF),
            "wv": wvs.astype(BF),
            "wp": w_proj[rows, :].astype(BF),
            "bq": bias_cols(bqs),
            "bk": bias_cols(bks),
            "bvb": np.broadcast_to(bvs, (128, GC)).copy(),
        })
    return maps


def kernel(x, w_qkv, b_qkv, w_proj, b_proj, _trace=False):
    import time
    from concourse import bass_utils
    nc = _get_nc()
    maps = _in_maps(x, w_qkv, b_qkv, w_proj, b_proj)
    try:
        res = bass_utils.run_bass_kernel_spmd(nc, maps,
                                              core_ids=list(range(N_CORES)),
                                              trace=_trace)
    except Exception:
        # a previously wedged device usually clears after one failed
        # attempt; retry once
        time.sleep(5)
        res = bass_utils.run_bass_kernel_spmd(nc, maps,
                                              core_ids=list(range(N_CORES)),
                                              trace=_trace)
    _CACHE["last_result"] = res
    b_proj = np.asarray(b_proj, np.float32)
    outs = np.empty((B, S, DIM), np.float32)
    for b in range(B):
        outs[b] = (res.results[2 * b]["out"] + res.results[2 * b + 1]["out"]
                   + b_proj)
    return outs
